# revision 13
# baseline (speedup 1.0000x reference)
"""Trainium2 Bass kernel for nn_DeepTransform (gnn_message_passing).

Strategy (8 NeuronCores, SPMD):
  - Edges sharded by DESTINATION node range: core r owns nodes
    [r*1250, (r+1)*1250) and all edges pointing into them. Segment-sum
    scatter therefore needs no cross-core reduction.
  - Node state (x, vec) lives sharded (token-major fp32 masters in SBUF).
  - Per layer: x_h (edge MLP of x) computed per-shard then AllGather'd as
    bf16 rows [N, 3H]; vec likewise AllGather'd as bf16 rows [N, 3H]
    (pre-scaled by 1/sqrt(H)). Per-edge gathers use gpsimd.dma_gather.
  - Scatter = one-hot matmul: edges sorted by destination, 128-edge tiles
    each targeting one 128-node block; S [128e,128n] one-hot and
    T_d = S * edge_vector_d / sqrt(H) ship from host; PSUM accumulates
    [128 nodes, 3H+H] per block.
  - MessageUpdating runs per 128-node block, feature-major via PE
    transposes of the token-major masters.
"""
import os
import sys

sys.path.insert(0, "/opt/trn_rl_repo")

import numpy as np
import ml_dtypes

from concourse import bass, bacc, mybir, tile, bass_utils
from concourse.masks import make_identity

BF16 = ml_dtypes.bfloat16

L, N, E, H, F = 3, 10000, 160000, 256, 256
NCORES = 8
NSH = N // NCORES            # 1250 nodes per core
P = 128
NBLK = (NSH + P - 1) // P    # 10 node blocks per core (last has 98 rows)
CH_TILES = 8                 # edge tiles per gather chunk
CHUNK = CH_TILES * P         # 1024 edges per chunk
H3 = 3 * H                   # 768
H4 = 4 * H                   # 1024

INV_SQRT3 = 1.0 / np.sqrt(3.0)
INV_SQRT2 = 1.0 / np.sqrt(2.0)
INV_SQRTH = 1.0 / np.sqrt(float(H))
SILU_SCALE = 1.0 / 0.6

FP32 = mybir.dt.float32
BF = mybir.dt.bfloat16
I16 = mybir.dt.int16
Act = mybir.ActivationFunctionType
Alu = mybir.AluOpType


# --------------------------------------------------------------------------
# Host-side preprocessing
# --------------------------------------------------------------------------

def _prep_edges(edge_index, edge_feat, edge_vector):
    """Shard edges by destination, sort, tile, build S/T/feat/idx arrays."""
    j = np.asarray(edge_index[0]).astype(np.int64)
    i = np.asarray(edge_index[1]).astype(np.int64)

    per_core = []
    tpb = 0
    for r in range(NCORES):
        sel = np.nonzero((i >= r * NSH) & (i < (r + 1) * NSH))[0]
        il = i[sel] - r * NSH
        order = np.argsort(il, kind="stable")
        eids = sel[order]
        il = il[order]
        blk = il // P
        counts = np.bincount(blk, minlength=NBLK)
        tpb = max(tpb, int(np.max((counts + P - 1) // P)))
        per_core.append((eids, il, blk, counts))

    TPB = tpb
    TILES = NBLK * TPB
    NCHUNKS = (TILES + CH_TILES - 1) // CH_TILES
    TILES_P = NCHUNKS * CH_TILES
    SLOTS = TILES_P * P

    ins = []
    for r in range(NCORES):
        eids, il, blk, counts = per_core[r]
        # slot assignment: block b occupies tiles [b*TPB, (b+1)*TPB)
        slot = np.full(len(eids), -1, dtype=np.int64)
        base = 0
        for b in range(NBLK):
            nb = counts[b]
            s0 = b * TPB * P
            slot[base:base + nb] = s0 + np.arange(nb)
            base += nb
        assert base == len(eids)

        jj = np.zeros(SLOTS, dtype=np.int16)
        destcol = np.full(SLOTS, -1, dtype=np.int64)
        jj[slot] = j[eids].astype(np.int16)
        destcol[slot] = il - blk * P

        # S / T one-hots, laid out [128 edge-part, TILES_P*128 (tile,nodecol)]
        S = np.zeros((P, TILES_P * P), dtype=BF16)
        T = np.zeros((3, P, TILES_P * P), dtype=BF16)
        valid = destcol >= 0
        vs = np.nonzero(valid)[0]
        t_of = vs // P
        e_of = vs % P
        col = destcol[vs]
        S[e_of, t_of * P + col] = np.float32(1.0)
        ev = np.asarray(edge_vector, dtype=np.float32)
        evv = np.zeros((SLOTS, 3), dtype=np.float32)
        evv[slot] = ev[eids]
        for d in range(3):
            T[d, e_of, t_of * P + col] = (evv[vs, d] * INV_SQRTH).astype(BF16)

        # edge features, feature-major lhsT tiles [2, 128 f, slots]
        ef = np.asarray(edge_feat, dtype=np.float32)
        featT = np.zeros((2, P, SLOTS), dtype=BF16)
        for k in range(2):
            featT[k][:, slot] = ef[eids, k * P:(k + 1) * P].T.astype(BF16)

        # gather indices, wrapped [chunk, 128, CHUNK//16] int16
        idx = np.zeros((NCHUNKS, P, CHUNK // 16), dtype=np.int16)
        for c in range(NCHUNKS):
            seg = jj[c * CHUNK:(c + 1) * CHUNK]
            w = seg.reshape(CHUNK // 16, 16).T  # [16, CHUNK//16]
            idx[c] = np.tile(w, (8, 1))

        ins.append(dict(feat=featT, S=S, T=T, idx=idx))

    return ins, TPB, TILES, NCHUNKS, TILES_P


def _prep_weights(mp_w1, mp_b1, mp_w2, mp_b2, mp_ew, mp_eb,
                  mu_vw, mu_w1, mu_b1, mu_w2, mu_b2):
    """Rearrange weights into lhsT/rhs tiles with constant folds."""
    w = {}
    mp_w1 = np.asarray(mp_w1, np.float32)
    w["w1"] = mp_w1.reshape(L, 2, P, P).astype(BF16)              # lhsT [l,k][128,128]
    w["b1"] = np.asarray(mp_b1, np.float32).reshape(L, P, 1)
    w["w2"] = (np.asarray(mp_w2, np.float32) * SILU_SCALE).astype(BF16)  # rhs [l][128,768]
    b2 = np.asarray(mp_b2, np.float32)
    w["b2_nonzero"] = bool(np.any(b2 != 0))
    w["b2rep"] = np.broadcast_to(b2[:, None, :], (L, P, H3)).astype(np.float32).copy()
    ew = np.asarray(mp_ew, np.float32) * INV_SQRT3
    w["ew"] = ew.reshape(L, 2, P, H3).astype(BF16)                # rhs [l,k][128,768]
    eb = np.asarray(mp_eb, np.float32) * INV_SQRT3
    w["eb_nonzero"] = bool(np.any(eb != 0))
    w["ebrep"] = np.broadcast_to(eb[:, None, :], (L, P, H3)).astype(np.float32).copy()
    w["vw"] = np.asarray(mu_vw, np.float32).reshape(L, 2, P, 2 * H).astype(BF16)  # rhs [l,k][128,512]
    # mu_w1 [512, 256] -> lhsT tiles [l, k(4), m(2), 128, 128]
    m1 = np.asarray(mu_w1, np.float32).reshape(L, 4, P, 2, P).transpose(0, 1, 3, 2, 4)
    w["m1"] = m1.astype(BF16)
    w["mb1"] = np.asarray(mu_b1, np.float32).reshape(L, 2, P, 1)
    m2 = (np.asarray(mu_w2, np.float32) * SILU_SCALE).reshape(L, 2, P, 6, P).transpose(0, 1, 3, 2, 4)
    w["m2"] = m2.astype(BF16)                                      # lhsT [l,k(2),m(6),128,128]
    w["mb2"] = np.asarray(mu_b2, np.float32).reshape(L, 6, P, 1)
    return w


# --------------------------------------------------------------------------
# Device program
# --------------------------------------------------------------------------

def _build(TPB, TILES, NCHUNKS, TILES_P, wmeta):
    nc = bacc.Bacc("TRN2", target_bir_lowering=False, debug=False,
                   num_devices=NCORES)

    SLOTS = TILES_P * P
    eb_nz = wmeta["eb_nonzero"]
    b2_nz = wmeta["b2_nonzero"]

    # ---------------- inputs ----------------
    t_xs = nc.dram_tensor("xs", [NBLK, P, H], FP32, kind="ExternalInput")
    t_feat = nc.dram_tensor("feat", [2, P, SLOTS], BF, kind="ExternalInput")
    t_S = nc.dram_tensor("S", [P, SLOTS], BF, kind="ExternalInput")
    t_T = nc.dram_tensor("T", [3, P, SLOTS], BF, kind="ExternalInput")
    t_idx = nc.dram_tensor("idx", [NCHUNKS, P, CHUNK // 16], I16, kind="ExternalInput")
    t_w1 = nc.dram_tensor("w1", [L, 2, P, P], BF, kind="ExternalInput")
    t_b1 = nc.dram_tensor("b1", [L, P, 1], FP32, kind="ExternalInput")
    t_w2 = nc.dram_tensor("w2", [L, P, H3], BF, kind="ExternalInput")
    t_b2rep = nc.dram_tensor("b2rep", [L, P, H3], FP32, kind="ExternalInput")
    t_ew = nc.dram_tensor("ew", [L, 2, P, H3], BF, kind="ExternalInput")
    t_ebrep = nc.dram_tensor("ebrep", [L, P, H3], FP32, kind="ExternalInput")
    t_vw = nc.dram_tensor("vw", [L, 2, P, 2 * H], BF, kind="ExternalInput")
    t_m1 = nc.dram_tensor("m1", [L, 4, 2, P, P], BF, kind="ExternalInput")
    t_mb1 = nc.dram_tensor("mb1", [L, 2, P, 1], FP32, kind="ExternalInput")
    t_m2 = nc.dram_tensor("m2", [L, 2, 6, P, P], BF, kind="ExternalInput")
    t_mb2 = nc.dram_tensor("mb2", [L, 6, P, 1], FP32, kind="ExternalInput")

    t_ox = nc.dram_tensor("out_x", [NBLK * P, H], FP32, kind="ExternalOutput")
    t_ov = nc.dram_tensor("out_vec", [NBLK * P, H3], FP32, kind="ExternalOutput")
    DBG_XH = bool(int(os.environ.get("TRN_DBG_XH", "0")))
    t_dxh = nc.dram_tensor("out_dbg_xh", [N, H3], BF, kind="ExternalOutput") if DBG_XH else None

    with tile.TileContext(nc) as tc:
        ctxs = []

        def pool(name, bufs, space="SBUF"):
            p = tc.tile_pool(name=name, bufs=bufs, space=space)
            ctxs.append(p)
            return p.__enter__()

        pconst = pool("pconst", 1)
        pmast = pool("pmast", 1)
        pwts = pool("pwts", 1)
        pgat = pool("pgat", 2)
        pedge = pool("pedge", 3)
        pmu = pool("pmu", 2)
        pmub = pool("pmub", 2)
        pdram = pool("pdram", 1, space="DRAM")
        ps_rbf = pool("ps_rbf", 2, space="PSUM")
        ps_acc = pool("ps_acc", 1, space="PSUM")
        ps_mu = pool("ps_mu", 2, space="PSUM")

        # identities for PE transpose
        ident_f = pconst.tile([P, P], FP32, name="ident_f")
        make_identity(nc, ident_f[:])
        ident_b = pconst.tile([P, P], BF, name="ident_b")
        nc.vector.tensor_copy(ident_b[:], ident_f[:])
        eps_c = pconst.tile([P, 1], FP32, name="eps_c")
        nc.gpsimd.memset(eps_c[:], 1e-8)

        # ---------------- persistent state ----------------
        xm = [pmast.tile([P, H], FP32, name=f"xm{b}") for b in range(NBLK)]
        vm = [pmast.tile([P, H3], FP32, name=f"vm{b}") for b in range(NBLK)]
        for b in range(NBLK):
            nc.sync.dma_start(xm[b][:], t_xs[b])
            nc.gpsimd.memset(vm[b][:], 0.0)

        # ---------------- weights (resident) ----------------
        w1 = [[pwts.tile([P, P], BF, name=f"w1_{l}_{k}") for k in range(2)] for l in range(L)]
        b1 = [pwts.tile([P, 1], FP32, name=f"b1_{l}") for l in range(L)]
        w2 = [pwts.tile([P, H3], BF, name=f"w2_{l}") for l in range(L)]
        ew = [[pwts.tile([P, H3], BF, name=f"ew_{l}_{k}") for k in range(2)] for l in range(L)]
        vw = [[pwts.tile([P, 2 * H], BF, name=f"vw_{l}_{k}") for k in range(2)] for l in range(L)]
        m1 = [[[pwts.tile([P, P], BF, name=f"m1_{l}_{k}_{m}") for m in range(2)]
               for k in range(4)] for l in range(L)]
        mb1 = [[pwts.tile([P, 1], FP32, name=f"mb1_{l}_{m}") for m in range(2)] for l in range(L)]
        m2 = [[[pwts.tile([P, P], BF, name=f"m2_{l}_{k}_{m}") for m in range(6)]
               for k in range(2)] for l in range(L)]
        mb2 = [[pwts.tile([P, 1], FP32, name=f"mb2_{l}_{m}") for m in range(6)] for l in range(L)]
        for l in range(L):
            for k in range(2):
                nc.sync.dma_start(w1[l][k][:], t_w1[l, k])
                nc.sync.dma_start(ew[l][k][:], t_ew[l, k])
                nc.sync.dma_start(vw[l][k][:], t_vw[l, k])
            nc.sync.dma_start(b1[l][:], t_b1[l])
            nc.sync.dma_start(w2[l][:], t_w2[l])
            for k in range(4):
                for m in range(2):
                    nc.sync.dma_start(m1[l][k][m][:], t_m1[l, k, m])
            for m in range(2):
                nc.sync.dma_start(mb1[l][m][:], t_mb1[l, m])
            for k in range(2):
                for m in range(6):
                    nc.sync.dma_start(m2[l][k][m][:], t_m2[l, k, m])
            for m in range(6):
                nc.sync.dma_start(mb2[l][m][:], t_mb2[l, m])
        ebrep = None
        b2rep = None
        if eb_nz:
            ebrep = [pwts.tile([P, H3], FP32, name=f"ebrep{l}") for l in range(L)]
            for l in range(L):
                nc.sync.dma_start(ebrep[l][:], t_ebrep[l])
        if b2_nz:
            b2rep = [pwts.tile([P, H3], FP32, name=f"b2rep{l}") for l in range(L)]
            for l in range(L):
                nc.sync.dma_start(b2rep[l][:], t_b2rep[l])

        # ---------------- DRAM comm buffers ----------------
        xh_sh = [pdram.tile([NSH, H3], BF, name=f"xh_sh{l}") for l in range(L)]
        xh_full = [pdram.tile([N, H3], BF, name=f"xh_full{l}", addr_space="Shared")
                   for l in range(L)]
        vr_sh = [pdram.tile([NSH, H3], BF, name=f"vr_sh{l}") for l in range(1, L)]
        vr_full = [pdram.tile([N, H3], BF, name=f"vr_full{l}", addr_space="Shared")
                   for l in range(1, L)]

        RG = [list(range(NCORES))]

        def rows_of(b):
            return min(P, NSH - b * P)

        def transpose_to(dst_sb, src_ap, ident, rows=P, cols=P):
            """dst_sb[c, r] = src_ap[r, c]; src partitions=rows."""
            pt = ps_mu.tile([P, P], FP32, name="pt", tag="ps_mu")
            nc.tensor.transpose(pt[:cols, :rows], src_ap, ident[:rows, :rows])
            nc.scalar.copy(dst_sb, pt[:cols, :rows])

        # ---------------- phase A: x_h for layer l, node block b ----------
        def phase_a(l, b):
            """Compute x_h rows for block b with layer-l edge-MLP weights,
            write into xh_sh[l]. Uses current x master."""
            rows = rows_of(b)
            xfm = []
            for k in range(2):
                xf = pmu.tile([P, P], BF, name=f"xfm{k}", tag=f"xfm{k}")
                transpose_to(xf[:, :rows], xm[b][:rows, k * P:(k + 1) * P], ident_f, rows=rows)
                xfm.append(xf)
            h1p = ps_mu.tile([P, P], FP32, name="h1p", tag="ps_mu")
            for k in range(2):
                nc.tensor.matmul(h1p[:, :rows], w1[l][k][:], xfm[k][:, :rows],
                                 start=(k == 0), stop=(k == 1))
            h1 = pmu.tile([P, P], BF, name="h1", tag="h1")
            nc.scalar.activation(h1[:, :rows], h1p[:, :rows], Act.Silu,
                                 bias=b1[l][:, :1], scale=1.0)
            xhp = ps_acc.tile([P, H3], FP32, name="xhp", tag="acc")
            nc.tensor.matmul(xhp[:rows, 0:512], h1[:, :rows], w2[l][:, 0:512],
                             start=True, stop=True)
            nc.tensor.matmul(xhp[:rows, 512:H3], h1[:, :rows], w2[l][:, 512:H3],
                             start=True, stop=True)
            xh = pmu.tile([P, H3], BF, name="xh", tag="xh")
            if b2_nz:
                xt = pmu.tile([P, H3], FP32, name="xt", tag="xh32")
                nc.vector.tensor_tensor(out=xt[:rows, :], in0=xhp[:rows, :],
                                        in1=b2rep[l][:rows, :], op=Alu.add)
                nc.vector.tensor_copy(xh[:rows, :], xt[:rows, :])
            else:
                nc.scalar.copy(xh[:rows, :], xhp[:rows, :])
            nc.sync.dma_start(xh_sh[l][b * P:b * P + rows, :], xh[:rows, :])

        # ---------------- mu phase (MessageUpdating) for block b ----------
        def phase_mu(l, b):
            rows = rows_of(b)
            # vecT tiles [d][hc]: [128 h, rows]
            vecT = []
            for d in range(3):
                row = []
                for hc in range(2):
                    vt = pmu.tile([P, P], BF, name=f"vecT{d}{hc}", tag=f"vecT{d}{hc}")
                    transpose_to(vt[:, :rows],
                                 vm[b][:rows, d * H + hc * P:d * H + (hc + 1) * P],
                                 ident_f, rows=rows)
                    row.append(vt)
                vecT.append(row)
            # vp[d][m] psum [128 c, rows] feature-major; vw tile is lhsT
            # (vec1: m 0..1, vec2: m 2..3)
            vp_sb = [[None] * 4 for _ in range(3)]
            for m in range(4):
                for d in range(3):
                    vpp = ps_mu.tile([P, P], FP32, name="vpp", tag="ps_mu")
                    for k in range(2):
                        nc.tensor.matmul(vpp[:, :rows], vw[l][k][:, m * P:(m + 1) * P],
                                         vecT[d][k][:, :rows],
                                         start=(k == 0), stop=(k == 1))
                    vs = pmu.tile([P, P], BF, name=f"vp{d}{m}", tag=f"vp{d}{m}")
                    nc.scalar.copy(vs[:, :rows], vpp[:, :rows])
                    vp_sb[d][m] = vs
            # vec_dot[hc] = sum_d vp[d][hc] * vp[d][hc+2]  (bf16, fm [c, rows])
            vdot, vnorm = [], []
            for hc in range(2):
                q = pmu.tile([P, P], FP32, name=f"q{hc}", tag=f"q{hc}")
                nc.vector.tensor_tensor(out=q[:, :rows], in0=vp_sb[0][hc][:, :rows],
                                        in1=vp_sb[0][hc + 2][:, :rows], op=Alu.mult)
                for d in (1, 2):
                    q2 = pmu.tile([P, P], FP32, name=f"q2_{hc}", tag=f"q2{hc}")
                    nc.vector.tensor_tensor(out=q2[:, :rows], in0=vp_sb[d][hc][:, :rows],
                                            in1=vp_sb[d][hc + 2][:, :rows], op=Alu.mult)
                    nc.vector.tensor_tensor(out=q[:, :rows], in0=q[:, :rows],
                                            in1=q2[:, :rows], op=Alu.add)
                vd = pmu.tile([P, P], BF, name=f"vdot{hc}", tag=f"vdot{hc}")
                nc.scalar.activation(vd[:, :rows], q[:, :rows], Act.Copy,
                                     scale=INV_SQRTH)
                vdot.append(vd)
                # vnorm from vec2 squares
                s = pmu.tile([P, P], FP32, name=f"s{hc}", tag=f"s{hc}")
                nc.vector.tensor_tensor(out=s[:, :rows], in0=vp_sb[0][hc + 2][:, :rows],
                                        in1=vp_sb[0][hc + 2][:, :rows], op=Alu.mult)
                for d in (1, 2):
                    s2 = pmu.tile([P, P], FP32, name=f"s2_{hc}", tag=f"s2{hc}")
                    nc.vector.tensor_tensor(out=s2[:, :rows], in0=vp_sb[d][hc + 2][:, :rows],
                                            in1=vp_sb[d][hc + 2][:, :rows], op=Alu.mult)
                    nc.vector.tensor_tensor(out=s[:, :rows], in0=s[:, :rows],
                                            in1=s2[:, :rows], op=Alu.add)
                vn = pmu.tile([P, P], BF, name=f"vn{hc}", tag=f"vn{hc}")
                nc.scalar.activation(vn[:, :rows], s[:, :rows], Act.Sqrt,
                                     bias=eps_c[:, :1], scale=1.0)
                vnorm.append(vn)
            # x feature-major (post scatter-update x)
            xfm = []
            for hc in range(2):
                xf = pmu.tile([P, P], BF, name=f"muxf{hc}", tag=f"muxf{hc}")
                transpose_to(xf[:, :rows], xm[b][:rows, hc * P:(hc + 1) * P],
                             ident_f, rows=rows)
                xfm.append(xf)
            cat = [xfm[0], xfm[1], vnorm[0], vnorm[1]]
            # g1 fm [m(2)][128 c1, rows]
            g1 = []
            for m in range(2):
                gp = ps_mu.tile([P, P], FP32, name="gp", tag="ps_mu")
                for k in range(4):
                    nc.tensor.matmul(gp[:, :rows], m1[l][k][m][:], cat[k][:, :rows],
                                     start=(k == 0), stop=(k == 3))
                g = pmu.tile([P, P], BF, name=f"g1_{m}", tag=f"g1{m}")
                nc.scalar.activation(g[:, :rows], gp[:, :rows], Act.Silu,
                                     bias=mb1[l][m][:, :1], scale=1.0)
                g1.append(g)
            # xh fm [m(6)][128 c2, rows] ; xv1=m0..1, xv2=m2..3, xv3=m4..5
            xv = []
            for m in range(6):
                xp = ps_mu.tile([P, P], FP32, name="xp", tag="ps_mu")
                for k in range(2):
                    nc.tensor.matmul(xp[:, :rows], m2[l][k][m][:], g1[k][:, :rows],
                                     start=(k == 0), stop=(k == 1))
                xs = pmu.tile([P, P], FP32, name=f"xv{m}", tag=f"xv{m}")
                nc.scalar.activation(xs[:, :rows], xp[:, :rows], Act.Identity,
                                     bias=mb2[l][m][:, :1], scale=1.0)
                xv.append(xs)
            # x update: x += (xv1 + xv2*vdot) * INV_SQRT2   (fm -> transpose)
            for hc in range(2):
                u = pmu.tile([P, P], FP32, name=f"u{hc}", tag=f"u{hc}")
                nc.vector.tensor_tensor(out=u[:, :rows], in0=xv[2 + hc][:, :rows],
                                        in1=vdot[hc][:, :rows], op=Alu.mult)
                nc.vector.tensor_tensor(out=u[:, :rows], in0=u[:, :rows],
                                        in1=xv[hc][:, :rows], op=Alu.add)
                ut = ps_mu.tile([P, P], FP32, name="ut", tag="ps_mu")
                nc.tensor.transpose(ut[:rows, :], u[:, :rows], ident_f)
                us = pmu.tile([P, P], FP32, name=f"us{hc}", tag=f"us{hc}")
                nc.scalar.activation(us[:rows, :], ut[:rows, :], Act.Copy,
                                     scale=INV_SQRT2)
                nc.vector.tensor_tensor(out=xm[b][:rows, hc * P:(hc + 1) * P],
                                        in0=xm[b][:rows, hc * P:(hc + 1) * P],
                                        in1=us[:rows, :], op=Alu.add)
            # vec update: vec += xv3 * vec1   (vec1 = vp[d][0..1])
            for d in range(3):
                for hc in range(2):
                    pr = pmu.tile([P, P], BF, name=f"pr{d}{hc}", tag=f"pr{hc}")
                    nc.vector.tensor_tensor(out=pr[:, :rows], in0=xv[4 + hc][:, :rows],
                                            in1=vp_sb[d][hc][:, :rows], op=Alu.mult)
                    prt = ps_mu.tile([P, P], BF, name="prt", tag="ps_mu")
                    nc.tensor.transpose(prt[:rows, :], pr[:, :rows], ident_b)
                    nc.vector.tensor_tensor(
                        out=vm[b][:rows, d * H + hc * P:d * H + (hc + 1) * P],
                        in0=vm[b][:rows, d * H + hc * P:d * H + (hc + 1) * P],
                        in1=prt[:rows, :], op=Alu.add)

        def write_vec_rm(l, b):
            """vec master rows -> vr_sh[l] (bf16, scaled by 1/sqrt(H))."""
            rows = rows_of(b)
            vr = pmu.tile([P, H3], BF, name="vr", tag="vr")
            nc.scalar.activation(vr[:rows, :], vm[b][:rows, :], Act.Copy,
                                 scale=INV_SQRTH)
            nc.sync.dma_start(vr_sh[l - 1][b * P:b * P + rows, :], vr[:rows, :])

        # ---------------- prologue: layer-0 x_h + AG ----------------
        for b in range(NBLK):
            phase_a(0, b)
        nc.gpsimd.collective_compute("AllGather", Alu.bypass, replica_groups=RG,
                                     ins=[xh_sh[0][:]], outs=[xh_full[0][:]])
        if DBG_XH:
            nc.sync.dma_start(t_dxh[:], xh_full[0][:])

        # ---------------- layers ----------------
        L_EFF = int(os.environ.get("TRN_LAYERS", str(L)))
        SKIP_MU = bool(int(os.environ.get("TRN_SKIP_MU", "0")))
        DBG_TSB = int(os.environ.get("TRN_DBG_TSB", "0"))
        for l in range(L_EFF):
            for c in range(NCHUNKS):
                feat_sb = []
                for k in range(2):
                    f = pgat.tile([P, CHUNK], BF, name=f"feat{k}", tag=f"feat{k}")
                    nc.sync.dma_start(f[:], t_feat[k, :, c * CHUNK:(c + 1) * CHUNK])
                    feat_sb.append(f)
                S_sb = pgat.tile([P, CHUNK], BF, name="S_sb", tag="S_sb")
                nc.sync.dma_start(S_sb[:], t_S[:, c * CHUNK:(c + 1) * CHUNK])
                T_sb = []
                for d in range(3):
                    td = pgat.tile([P, CHUNK], BF, name=f"T{d}", tag=f"T{d}")
                    nc.sync.dma_start(td[:], t_T[d, :, c * CHUNK:(c + 1) * CHUNK])
                    T_sb.append(td)
                idx_sb = pgat.tile([P, CHUNK // 16], I16, name="idx_sb", tag="idx")
                nc.sync.dma_start(idx_sb[:], t_idx[c])
                xg = pgat.tile([P, CH_TILES, H3], BF, name="xg", tag="xg")
                nc.gpsimd.dma_gather(xg[:], xh_full[l][:], idx_sb[:], CHUNK, CHUNK, H3)
                if l > 0:
                    vg = pgat.tile([P, CH_TILES, H3], BF, name="vg", tag="vg")
                    nc.gpsimd.dma_gather(vg[:], vr_full[l - 1][:], idx_sb[:],
                                         CHUNK, CHUNK, H3)

                for tl in range(CH_TILES):
                    t = c * CH_TILES + tl
                    if t >= TILES:
                        continue
                    b = t // TPB
                    first = (t % TPB == 0)
                    last = (t % TPB == TPB - 1)
                    e0 = tl * P

                    rp = ps_rbf.tile([P, H3], FP32, name="rp", tag="rbf")
                    for k in range(2):
                        nc.tensor.matmul(rp[:, 0:512], feat_sb[k][:, e0:e0 + P],
                                         ew[l][k][:, 0:512], start=(k == 0), stop=(k == 1))
                    for k in range(2):
                        nc.tensor.matmul(rp[:, 512:H3], feat_sb[k][:, e0:e0 + P],
                                         ew[l][k][:, 512:H3], start=(k == 0), stop=(k == 1))
                    tsb = pedge.tile([P, H3], BF, name="tsb", tag="tsb")
                    if eb_nz:
                        r32 = pedge.tile([P, H3], FP32, name="r32", tag="r32")
                        nc.vector.tensor_tensor(out=r32[:], in0=rp[:], in1=ebrep[l][:],
                                                op=Alu.add)
                        nc.vector.tensor_tensor(out=tsb[:], in0=r32[:],
                                                in1=xg[:, tl, :], op=Alu.mult)
                    elif DBG_TSB == 1:
                        nc.vector.tensor_copy(tsb[:], xg[:, tl, :])
                    elif DBG_TSB == 2:
                        nc.vector.tensor_copy(tsb[:], rp[:])
                    else:
                        nc.vector.tensor_tensor(out=tsb[:], in0=rp[:],
                                                in1=xg[:, tl, :], op=Alu.mult)

                    if l > 0:
                        pay = pedge.tile([P, H3], BF, name="pay", tag="pay")
                        for d in range(3):
                            nc.vector.tensor_tensor(
                                out=pay[:, d * H:(d + 1) * H], in0=tsb[:, 0:H],
                                in1=vg[:, tl, d * H:(d + 1) * H], op=Alu.mult)

                    if first:
                        acc = ps_acc.tile([P, H4], FP32, name="acc", tag="acc")
                        nc.scalar.memzero(acc[:])
                        acc_cur[0] = acc
                    acc = acc_cur[0]
                    S_t = S_sb[:, e0:e0 + P]
                    if l > 0:
                        nc.tensor.matmul(acc[:, 0:512], S_t, pay[:, 0:512],
                                         start=False, stop=False, skip_group_check=True)
                        nc.tensor.matmul(acc[:, 512:H3], S_t, pay[:, 512:H3],
                                         start=False, stop=False, skip_group_check=True)
                    nc.tensor.matmul(acc[:, H3:H4], S_t, tsb[:, 512:H3],
                                     start=False, stop=last, skip_group_check=True)
                    for d in range(3):
                        nc.tensor.matmul(acc[:, d * H:(d + 1) * H],
                                         T_sb[d][:, e0:e0 + P], tsb[:, H:2 * H],
                                         start=False,
                                         stop=(last and d == 2), skip_group_check=True)

                    if last:
                        rows = rows_of(b)
                        # x = (x + d_x) * INV_SQRT2 ; vec += d_vec
                        nc.vector.tensor_tensor(out=xm[b][:rows, :], in0=xm[b][:rows, :],
                                                in1=acc[:rows, H3:H4], op=Alu.add)
                        nc.scalar.activation(xm[b][:rows, :], xm[b][:rows, :],
                                             Act.Copy, scale=INV_SQRT2)
                        nc.vector.tensor_tensor(out=vm[b][:rows, :], in0=vm[b][:rows, :],
                                                in1=acc[:rows, 0:H3], op=Alu.add)
                        if not SKIP_MU:
                            phase_mu(l, b)
                        if l < L_EFF - 1:
                            phase_a(l + 1, b)
                            write_vec_rm(l + 1, b)
                        else:
                            nc.sync.dma_start(t_ox[b * P:b * P + rows, :], xm[b][:rows, :])
                            nc.sync.dma_start(t_ov[b * P:b * P + rows, :], vm[b][:rows, :])
            if l < L_EFF - 1:
                nc.gpsimd.collective_compute("AllGather", Alu.bypass, replica_groups=RG,
                                             ins=[xh_sh[l + 1][:]], outs=[xh_full[l + 1][:]])
                nc.gpsimd.collective_compute("AllGather", Alu.bypass, replica_groups=RG,
                                             ins=[vr_sh[l][:]], outs=[vr_full[l][:]])

        for p in reversed(ctxs):
            p.__exit__(None, None, None)

    nc.compile()
    return nc


acc_cur = [None]


# --------------------------------------------------------------------------
# Entry point
# --------------------------------------------------------------------------

def kernel(x, edge_feat, edge_vector, edge_index,
           mp_w1, mp_b1, mp_w2, mp_b2, mp_ew, mp_eb,
           mu_vw, mu_w1, mu_b1, mu_w2, mu_b2):
    per_core, TPB, TILES, NCHUNKS, TILES_P = _prep_edges(edge_index, edge_feat,
                                                         edge_vector)
    w = _prep_weights(mp_w1, mp_b1, mp_w2, mp_b2, mp_ew, mp_eb,
                      mu_vw, mu_w1, mu_b1, mu_w2, mu_b2)

    nc = _build(TPB, TILES, NCHUNKS, TILES_P, w)

    x = np.asarray(x, np.float32)
    xs_pad = np.zeros((NCORES, NBLK * P, H), np.float32)
    for r in range(NCORES):
        xs_pad[r, :NSH] = x[r * NSH:(r + 1) * NSH]

    shared = dict(
        w1=w["w1"], b1=w["b1"], w2=w["w2"], b2rep=w["b2rep"], ew=w["ew"],
        ebrep=w["ebrep"], vw=w["vw"], m1=w["m1"], mb1=w["mb1"], m2=w["m2"],
        mb2=w["mb2"],
    )
    in_maps = []
    for r in range(NCORES):
        m = dict(shared)
        m["xs"] = xs_pad[r].reshape(NBLK, P, H)
        m["feat"] = per_core[r]["feat"]
        m["S"] = per_core[r]["S"]
        m["T"] = per_core[r]["T"]
        m["idx"] = per_core[r]["idx"]
        in_maps.append(m)

    res = bass_utils.run_bass_kernel_spmd(nc, in_maps, core_ids=list(range(NCORES)))
    kernel.last_exec_time_ns = res.exec_time_ns
    kernel.last_ctx = (nc, in_maps)

    xo = np.zeros((N, H), np.float32)
    vo = np.zeros((N, 3, H), np.float32)
    for r in range(NCORES):
        xo[r * NSH:(r + 1) * NSH] = res.results[r]["out_x"][:NSH]
        vo[r * NSH:(r + 1) * NSH] = res.results[r]["out_vec"][:NSH].reshape(NSH, 3, H)
    return xo, vo


kernel.last_exec_time_ns = None
kernel.last_ctx = None


def time_exec(nc, in_maps, iters=5):
    """Time device execution (s) via the PJRT path, inputs device-resident,
    no donation. Returns (min_s, all_times)."""
    import time
    import jax
    from jax.sharding import Mesh, PartitionSpec, NamedSharding
    from jax.experimental.shard_map import shard_map
    from concourse import bass2jax, mybir as mb

    bass2jax.install_neuronx_cc_hook()
    n_cores = len(in_maps)
    partition_name = nc.partition_id_tensor.name if nc.partition_id_tensor else None
    in_names, out_names, out_avals = [], [], []
    for alloc in nc.m.functions[0].allocations:
        if not isinstance(alloc, mb.MemoryLocationSet):
            continue
        name = alloc.memorylocations[0].name
        if alloc.kind == "ExternalInput":
            if name != partition_name:
                in_names.append(name)
        elif alloc.kind == "ExternalOutput":
            out_names.append(name)
            out_avals.append(jax.core.ShapedArray(tuple(alloc.tensor_shape),
                                                  mb.dt.np(alloc.dtype)))
    n_params = len(in_names)
    all_names = in_names + out_names
    if partition_name is not None:
        all_names.append(partition_name)

    def _body(*args):
        operands = list(args)
        if partition_name is not None:
            operands.append(bass2jax.partition_id_tensor())
        return tuple(bass2jax._bass_exec_p.bind(
            *operands, out_avals=tuple(out_avals),
            in_names=tuple(all_names), out_names=tuple(out_names),
            lowering_input_output_aliases=(),
            sim_require_finite=True, sim_require_nnan=True, nc=nc))

    devices = jax.devices()[:n_cores]
    mesh = Mesh(np.array(devices), ("core",))
    spec = PartitionSpec("core")
    fn = jax.jit(shard_map(_body, mesh=mesh,
                           in_specs=(spec,) * (n_params + len(out_names)),
                           out_specs=(spec,) * len(out_names), check_rep=False),
                 keep_unused=True)
    sh = NamedSharding(mesh, spec)
    concat_in = [jax.device_put(
        np.concatenate([np.asarray(in_maps[c][nm]) for c in range(n_cores)], axis=0), sh)
        for nm in in_names]
    zeros = [jax.device_put(
        np.zeros((n_cores * a.shape[0], *a.shape[1:]), a.dtype), sh)
        for a in out_avals]
    # warm-up (NEFF load etc.)
    jax.block_until_ready(fn(*concat_in, *zeros))
    times = []
    for _ in range(iters):
        t0 = time.perf_counter()
        jax.block_until_ready(fn(*concat_in, *zeros))
        times.append(time.perf_counter() - t0)
    return min(times), times
